# revision 8
# baseline (speedup 1.0000x reference)
"""Adaptive-threshold spike encoding on 8 TRN2 NeuronCores.

Math: reference scans t=0..31 with
    acc += x; spike = acc >= thr_t; acc = spike ? 0 : acc; thr' = 0.9*thr + 0.1*|x|
With thr_t = x + 0.9^t*(0.5-x) (closed form), spike_t <=> acc_pre >= 0.9^t*(0.5-x)
where acc_pre = k*x (k = steps since last reset).  Dividing by x and scaling by
g^t (g = 1/0.9):  spike_t <=> m >= r  with  r = (0.5-x)/x  and  m = k*g^t,
updated as m' = (m*g + g^(t+1)) * (1 - spike).

Per core (feature-sharded, 8192 features each): ScalarE computes the affine
update M = g*m + q_t, VectorE does the compare (uint8 "no-spike" out) and the
reset multiply.  Host flips uint8 ns -> f32 spikes.
"""

import numpy as np
import concourse.bass as bass
import concourse.mybir as mybir
from concourse.bass_utils import run_bass_kernel_spmd

B = 32
F = 65536
T = 32
NCORES = 8
FS = F // NCORES  # 8192 features per core
FH = 4
FL = FS // FH  # 2048
P = B * FH  # 128 partitions

G = 1.0 / 0.9
NS_RING = 4

_cache: dict = {}


def _build(repeat: int = 1) -> bass.Bass:
    f32 = mybir.dt.float32
    u8 = mybir.dt.uint8
    ALU = mybir.AluOpType
    ACTF = mybir.ActivationFunctionType

    S = T * repeat  # global steps

    nc = bass.Bass(target_bir_lowering=False)
    x = nc.declare_dram_parameter("x", [B, FS], f32, isOutput=False)
    out = nc.declare_dram_parameter("out", [B, T, FS], u8, isOutput=True)

    with (
        nc.sbuf_tensor("x_sb", [P, FL], f32) as x_sb,
        nc.sbuf_tensor("d_sb", [P, FL], f32) as d_sb,
        nc.sbuf_tensor("inv_sb", [P, FL], f32) as inv_sb,
        nc.sbuf_tensor("r_sb", [P, FL], f32) as r_sb,
        nc.sbuf_tensor("m0", [P, FL], f32) as m0,
        nc.sbuf_tensor("m1", [P, FL], f32) as m1,
        nc.sbuf_tensor("mm0", [P, FL], f32) as mm0,
        nc.sbuf_tensor("mm1", [P, FL], f32) as mm1,
        nc.sbuf_tensor("ns0", [P, FL], u8) as ns0,
        nc.sbuf_tensor("ns1", [P, FL], u8) as ns1,
        nc.sbuf_tensor("ns2", [P, FL], u8) as ns2,
        nc.sbuf_tensor("ns3", [P, FL], u8) as ns3,
        nc.semaphore("sem_in") as sem_in,
        nc.semaphore("sem_M") as sem_M,
        nc.semaphore("sem_ns") as sem_ns,
        nc.semaphore("sem_mult") as sem_mult,
        nc.semaphore("sem_out") as sem_out,
        nc.Block() as block,
    ):
        xv = x[:, :].rearrange("b (fh fl) -> (b fh) fl", fh=FH)
        ov = out[:, :, :].rearrange("b t (fh fl) -> t b fh fl", fh=FH)
        ms = [m0, m1]
        mms = [mm0, mm1]
        nss = [ns0, ns1, ns2, ns3]

        # q for global step s uses local t = s % T
        qs = [float((1.0 / 0.9) ** ((s % T) + 1)) for s in range(S)]

        # Every step except the global last does exactly one sem_mult inc
        # (mult, or memset at rep boundaries); setup memset adds one more.
        # So sem_mult before step s == s + 1.  ACT skips boundary steps, so
        # ACT emissions through step s == s + 1 - s // T.
        last = S - 1

        @block.sync
        def _(sync):
            sync.dma_start(out=x_sb[:, :], in_=xv).then_inc(sem_in, 16)
            for s in range(S):
                sync.wait_ge(sem_ns, s + 1)
                sync.dma_start(
                    out=ov[s % T], in_=nss[s % NS_RING][:, :]
                ).then_inc(sem_out, 16)

        @block.scalar
        def _(scalar):
            for s in range(S):
                if s % T == T - 1:
                    continue  # boundary/final steps need no m update
                scalar.wait_ge(sem_mult, s + 1)
                scalar.activation(
                    mms[s % 2][:, :],
                    ms[s % 2][:, :],
                    ACTF.Copy,
                    bias=qs[s],
                    scale=G,
                )
                scalar.drain().then_inc(sem_M, 1)

        @block.vector
        def _(vector):
            # setup: r = (0.5 - x) / x, m0 = 0
            vector.wait_ge(sem_in, 16)
            vector.tensor_scalar(
                d_sb[:, :], x_sb[:, :], -1.0, 0.5, ALU.mult, ALU.add
            )
            vector.reciprocal(inv_sb[:, :], x_sb[:, :])
            vector.tensor_tensor(r_sb[:, :], d_sb[:, :], inv_sb[:, :], ALU.mult)
            vector.memset(m0[:, :], 0.0)
            vector.drain().then_inc(sem_mult, 1)

            for s in range(S):
                if s >= NS_RING:
                    vector.wait_ge(sem_out, 16 * (s - (NS_RING - 1)))
                vector.tensor_tensor(
                    nss[s % NS_RING][:, :], ms[s % 2][:, :], r_sb[:, :], ALU.is_lt
                )
                vector.drain().then_inc(sem_ns, 1)
                if s == last:
                    break
                if s % T == T - 1:
                    # rep boundary: restart the recurrence
                    vector.memset(ms[(s + 1) % 2][:, :], 0.0)
                    vector.drain().then_inc(sem_mult, 1)
                    continue
                vector.wait_ge(sem_M, s + 1 - s // T)
                vector.tensor_tensor(
                    ms[(s + 1) % 2][:, :],
                    mms[s % 2][:, :],
                    nss[s % NS_RING][:, :],
                    ALU.mult,
                )
                vector.drain().then_inc(sem_mult, 1)

    return nc


def _get_nc(repeat: int = 1) -> bass.Bass:
    if repeat not in _cache:
        _cache[repeat] = _build(repeat)
    return _cache[repeat]


def _run(x: np.ndarray, repeat: int = 1):
    nc = _get_nc(repeat)
    shards = [
        np.ascontiguousarray(x[:, i * FS : (i + 1) * FS]) for i in range(NCORES)
    ]
    in_maps = [{"x": s} for s in shards]
    res = run_bass_kernel_spmd(nc, in_maps, core_ids=list(range(NCORES)))
    return [r["out"] for r in res.results]


def kernel(x: np.ndarray) -> np.ndarray:
    outs = _run(np.asarray(x, dtype=np.float32), repeat=1)
    ns = np.concatenate(outs, axis=2)  # [B, T, F] uint8, 1 = no spike
    return (ns == 0).astype(np.float32)


# revision 10
# speedup vs baseline: 2.9689x; 2.9689x over previous
"""Adaptive-threshold spike encoding on 8 TRN2 NeuronCores.

Math: the reference scans t=0..31 with
    acc += x; spike = acc >= thr_t; acc = spike ? 0 : acc; thr' = 0.9*thr + 0.1*|x|
With thr_t = x + 0.9^t*(0.5-x) (closed form), spike_t <=> acc_pre >= 0.9^t*(0.5-x)
where acc_pre = k*x (k = steps since last reset).  Dividing by x and scaling by
g^t (g = 1/0.9):  spike_t <=> m >= r  with  r = (0.5-x)/x  and  m = k*g^t.

The whole step is ONE fused custom-DVE op:
    m' = select(m < r, g*m + q_t, 0)       (q_t = g^(t+1))
Since g*m + q_t >= g > 1 always, m' == 0  <=>  spike.  ScalarE turns the
state into the output byte (Sign: 0 -> 0, positive -> 1) and the uint8
"no-spike" plane is DMA'd out per step.  Host flips u8 -> f32 spikes.

Sharding: feature dim across the 8 cores, 8192 features each, no comms.
"""

import numpy as np
import concourse.bass as bass
import concourse.bacc as bacc
import concourse.mybir as mybir
from concourse import dve_ops as _dve_ops
from concourse.dve_spec import C0, C1, Spec, Src0, Src1, Zero, select
from concourse.bass_utils import run_bass_kernel_spmd

B = 32
F = 65536
T = 32
NCORES = 8
FS = F // NCORES  # 8192 features per core
FH = 4
FL = FS // FH  # 2048
P = B * FH  # 128 partitions

G = 1.0 / 0.9
RING = 4

_cache: dict = {}


def _spike_op():
    name = "SPIKE_STEP_ANT"
    for op in _dve_ops.OPS:
        if op.name == name:
            return op
    op = _dve_ops.DveOp(
        name,
        Spec(
            body=select(Src0 < Src1, Src0 * C0 + C1, Zero),
            reference=lambda in0, in1, s0, s1, imm2: np.where(
                in0 < in1, in0.astype(np.float32) * s0 + s1, 0.0
            ).astype(np.float32),
        ),
        subdim=False,
        uops_sha={"v3": "63ebce0dea841fb5", "v4": "520d0232c96c89cf"},
    )
    _dve_ops.OPS.append(op)
    _dve_ops.CUSTOM_DVE_SPECS[name] = op.spec
    _dve_ops._SUB_OPCODE_FOR_NAME[name] = (
        _dve_ops._CUSTOM_DVE_ROW_BASE + len(_dve_ops.OPS) - 1
    )
    return op


def _build(repeat: int = 1) -> bass.Bass:
    f32 = mybir.dt.float32
    u8 = mybir.dt.uint8
    ALU = mybir.AluOpType
    ACTF = mybir.ActivationFunctionType
    op = _spike_op()

    S = T * repeat  # global steps

    nc = bacc.Bacc(target_bir_lowering=False)
    x = nc.declare_dram_parameter("x", [B, FS], f32, isOutput=False)
    out = nc.declare_dram_parameter("out", [B, T, FS], u8, isOutput=True)

    with (
        nc.sbuf_tensor("x_sb", [P, FL], f32) as x_sb,
        nc.sbuf_tensor("d_sb", [P, FL], f32) as d_sb,
        nc.sbuf_tensor("inv_sb", [P, FL], f32) as inv_sb,
        nc.sbuf_tensor("r_sb", [P, FL], f32) as r_sb,
        nc.sbuf_tensor("mt0", [P, FL], f32) as mt0,
        nc.sbuf_tensor("mt1", [P, FL], f32) as mt1,
        nc.sbuf_tensor("mt2", [P, FL], f32) as mt2,
        nc.sbuf_tensor("mt3", [P, FL], f32) as mt3,
        nc.sbuf_tensor("sg0", [P, FL], u8) as sg0,
        nc.sbuf_tensor("sg1", [P, FL], u8) as sg1,
        nc.sbuf_tensor("sg2", [P, FL], u8) as sg2,
        nc.sbuf_tensor("sg3", [P, FL], u8) as sg3,
        nc.semaphore("sem_in") as sem_in,
        nc.semaphore("sem_m") as sem_m,
        nc.semaphore("sem_sg") as sem_sg,
        nc.semaphore("sem_out") as sem_out,
        nc.Block() as block,
    ):
        xv = x[:, :].rearrange("b (fh fl) -> (b fh) fl", fh=FH)
        ov = out[:, :, :].rearrange("b t (fh fl) -> t b fh fl", fh=FH)
        mts = [mt0, mt1, mt2, mt3]
        sgs = [sg0, sg1, sg2, sg3]

        qs = [float((1.0 / 0.9) ** ((s % T) + 1)) for s in range(S)]
        last = S - 1

        @block.sync
        def _(sync):
            sync.dma_start(out=x_sb[:, :], in_=xv).then_inc(sem_in, 16)
            for s in range(S):
                sync.wait_ge(sem_sg, s + 1)
                sync.dma_start(
                    out=ov[s % T], in_=sgs[s % RING][:, :]
                ).then_inc(sem_out, 16)

        @block.scalar
        def _(scalar):
            for s in range(S):
                scalar.wait_ge(sem_m, s + 1)
                if s >= RING:
                    scalar.wait_ge(sem_out, 16 * (s - (RING - 1)))
                scalar.activation(
                    sgs[s % RING][:, :], mts[(s + 1) % RING][:, :], ACTF.Sign
                )
                scalar.drain().then_inc(sem_sg, 1)

        @block.vector
        def _(vector):
            # setup: r = (0.5 - x) / x, m = 0 (no cross-engine consumers)
            vector.wait_ge(sem_in, 16)
            vector.tensor_scalar(
                d_sb[:, :], x_sb[:, :], -1.0, 0.5, ALU.mult, ALU.add
            )
            vector.reciprocal_approx_accurate(
                inv_sb[:, :], x_sb[:, :], scratch=mt1[:, :]
            )
            vector.tensor_tensor(r_sb[:, :], d_sb[:, :], inv_sb[:, :], ALU.mult)
            vector.tensor_scalar(
                mt0[:, :], x_sb[:, :], 0.0, None, ALU.mult
            )
            vector.drain()

            for s in range(S):
                if s >= RING:
                    # mt[(s+1)%RING] was read by ACT at step s-RING
                    vector.wait_ge(sem_sg, s - (RING - 1))
                vector._custom_dve(
                    op,
                    out=mts[(s + 1) % RING][:, :],
                    in0=mts[s % RING][:, :],
                    in1=r_sb[:, :],
                    s0=G,
                    s1=qs[s],
                    imm2=0.0,
                )
                vector.drain().then_inc(sem_m, 1)
                if s % T == T - 1 and s != last:
                    # rep boundary (timing builds): restart the recurrence,
                    # but only after ACT consumed this step's state.
                    vector.wait_ge(sem_sg, s + 1)
                    vector.tensor_scalar(
                        mts[(s + 1) % RING][:, :], x_sb[:, :], 0.0, None, ALU.mult
                    )
                    vector.drain()

    nc.finalize()
    return nc


def _get_nc(repeat: int = 1) -> bass.Bass:
    if repeat not in _cache:
        _cache[repeat] = _build(repeat)
    return _cache[repeat]


def _run(x: np.ndarray, repeat: int = 1):
    nc = _get_nc(repeat)
    shards = [
        np.ascontiguousarray(x[:, i * FS : (i + 1) * FS]) for i in range(NCORES)
    ]
    in_maps = [{"x": s} for s in shards]
    res = run_bass_kernel_spmd(nc, in_maps, core_ids=list(range(NCORES)))
    return [r["out"] for r in res.results]


def kernel(x: np.ndarray) -> np.ndarray:
    outs = _run(np.asarray(x, dtype=np.float32), repeat=1)
    ns = np.concatenate(outs, axis=2)  # [B, T, F] uint8, 1 = no spike
    return (ns == 0).astype(np.float32)
